# revision 31
# baseline (speedup 1.0000x reference)
"""Trainium2 Bass kernel for nn_BMManager: Linear([B,S,1024]->[B,S,512]) + bias,
then per-row segment forward-fill (expand_goals).

v3 strategy (data-parallel over batch, 8 cores x 4 batch rows each):

  out[t] = y[idx(t)], y = x @ W^T + b. With a p=0.5 mask only ~half the rows
  are distinct segment starts, so the GEMM runs on COMPACT rows only. No
  device-side gathers (v1 spent 262us of serial GpSimd descriptor-gen):

  Host (numpy): computes the forward-fill index and re-slots the compact
  rows into a *common single-chunk window schedule* shared by all 8 cores:
  every 128-t output tile's sources are placed inside ONE 128-slot chunk
  cc[ti] baked into the program (always feasible: a tile references at most
  128 distinct sources; boundary-shared sources are duplicated, lagging
  cores pad). Uploads x compact pre-transposed bf16 ([1024, j_pad],
  ~18MB/core), W^T bf16, bias broadcast f32, and the per-t relative source
  rank (srcrank_rel in [0,128)) pre-broadcast across partitions in bf16.

  Device, per core:
   1. dense HWDGE DMA loads of x^T (512KB pair-chunk tiles, sync queue).
   2. compact GEMM: per 128-slot chunk, 8 accumulating bf16 matmuls
      (lhsT = x^T slice, rhs = W^T tile) -> PSUM f32; DVE adds bias and
      casts into resident bf16 yc [128, nchunk, 512].
   3. expansion as one-hot matmul: out_tile[t, g] = sum_j E[j, t] yc[j, g],
      exactly ONE matmul per tile (rhs = yc chunk cc[ti]). E built on-device:
      is_equal(srcrank_rel_bcast, iota) on DVE (bf16 2x), one op per 512-t.
   4. DVE/ACT copy PSUM -> bf16 SBUF staging; 512KB stores on the scalar
      HWDGE queue (so stores never head-of-line-block x loads on sync).
      Host upcasts bf16 -> f32.

  PE stream: ~(nchunk*8 + 128) N=512 bf16 matmuls back-to-back (~145us),
  all with FWL-eligible 128-col weights; HAM stays warm.
"""

import numpy as np
import ml_dtypes

import concourse.bacc as bacc
import concourse.mybir as mybir
import concourse.tile as tile
from concourse.bass_utils import run_bass_kernel_spmd

P = 128
N_CORES = 8
B_FULL, S, D_IN, D_GOAL = 32, 4096, 1024, 512
B_PC = B_FULL // N_CORES          # 4 batch rows per core
R = B_PC * S                      # 16384 output rows per core
K_TILES = D_IN // P               # 8
NT = R // P                       # 128 output tiles per core
NB = NT // 4                      # 32 E-build blocks (512 t each)

F32 = mybir.dt.float32
BF16 = mybir.dt.bfloat16
BF = ml_dtypes.bfloat16

EXPAND_SLACK = 4                  # chunks of slack before emitting a tile


def ts(i, n):
    return slice(i * n, (i + 1) * n)


# ---------------------------------------------------------------- host side
def _ffill_index(critic_mask_core):
    """Forward-fill source index per flattened t for one core's 4 rows."""
    mc = np.asarray(critic_mask_core).astype(bool)        # [4, S]
    cond = np.ones((B_PC, S), dtype=bool)
    cond[:, 1:] = mc[:, :-1]
    condf = cond.reshape(-1)                              # [R]
    sel = np.where(condf, np.arange(R), -1)
    idx = np.maximum.accumulate(sel)                      # [R]
    return condf, idx


def _greedy(idx, condf, cc):
    """Place this core's sources into the common window schedule cc.

    All sources of tile ti must land in slots [128*cc[ti], 128*cc[ti]+256).
    Returns (None, (slots_src, srcslot)) on success or (ti, None) on
    overflow.
    """
    srcslot = np.empty(R, np.int64)
    slots_src = []
    s = 0
    last_src = -1
    last_slot = -1
    for ti in range(NT):
        w_lo = P * cc[ti]
        w_hi = w_lo + 2 * P
        if s < w_lo:
            slots_src.extend([0] * (w_lo - s))
            s = w_lo
        t0 = ti * P
        iv = idx[t0 : t0 + P]
        cv = condf[t0 : t0 + P]
        r0 = int(iv[0])
        carried_slot = -1
        if r0 < t0:
            if r0 == last_src and last_slot >= w_lo:
                carried_slot = last_slot
            else:                              # re-place (duplicate) in window
                if s >= w_hi:
                    return ti, None
                carried_slot = s
                slots_src.append(r0)
                s += 1
        new_rs = t0 + np.nonzero(cv)[0]
        k = len(new_rs)
        if s + k > w_hi:
            return ti, None
        base = s
        slots_src.extend(new_rs.tolist())
        s += k
        pos = np.searchsorted(new_rs, iv)
        srcslot[t0 : t0 + P] = np.where(iv < t0, carried_slot, base + pos)
        if k:
            last_src = int(new_rs[-1])
            last_slot = base + k - 1
        elif carried_slot >= 0:
            last_src = r0
            last_slot = carried_slot
    return None, (np.array(slots_src, np.int64), srcslot)


def _schedule(cores):
    """Common two-chunk window schedule cc[ti] + per-core placements."""
    cc = np.zeros(NT, np.int64)
    for condf, idx in cores:
        srcrank = np.cumsum(condf) - 1
        lo = srcrank[idx[np.arange(NT) * P]] // P
        cc = np.maximum(cc, lo)
    cc = np.maximum.accumulate(cc)
    for _ in range(500):
        placements = []
        bad = -1
        for condf, idx in cores:
            ov, res = _greedy(idx, condf, cc)
            if ov is not None:
                bad = max(bad, ov)
                break
            placements.append(res)
        if bad < 0:
            return cc, placements
        cc[bad] += 1
        cc = np.maximum.accumulate(cc)
    raise RuntimeError("window schedule failed to converge")


def _host_prep(x, critic_mask):
    cores = [
        _ffill_index(critic_mask[c * B_PC : (c + 1) * B_PC]) for c in range(N_CORES)
    ]
    cc, placements = _schedule(cores)
    nchunk = int(cc.max()) + 2
    nchunk = -(-nchunk // 2) * 2                       # keep x loads in pairs
    j_pad = nchunk * P

    straddle = np.zeros(NT, bool)
    for _, srcslot in placements:
        hi = srcslot.reshape(NT, P).max(axis=1)
        straddle |= hi >= (cc + 1) * P

    x = np.asarray(x)
    in_maps = []
    for c in range(N_CORES):
        slots_src, srcslot = placements[c]
        slots = np.zeros(j_pad, np.int64)
        slots[: slots_src.size] = slots_src
        xf = x[c * B_PC : (c + 1) * B_PC].reshape(R, D_IN)
        xc = xf[slots]                                  # [j_pad, 1024] f32
        xcT = np.ascontiguousarray(xc.T).astype(BF)     # [1024, j_pad] bf16
        rel = (srcslot - P * cc[np.arange(R) // P]).astype(np.float32)
        assert rel.min() >= 0 and rel.max() < 2 * P
        srel = np.ascontiguousarray(
            np.broadcast_to(rel.astype(BF)[None, :], (P, R))
        )
        in_maps.append({"xT": xcT, "srel": srel})
    return cc, straddle, nchunk, in_maps


# -------------------------------------------------------------- device side
def build_program(nchunk, cc, straddle):
    cc = list(cc)
    straddle = list(straddle)
    j_pad = nchunk * P
    npairs = nchunk // 2
    NSREL = 8                                          # srel load pieces
    nc = bacc.Bacc(
        "TRN2",
        target_bir_lowering=False,
        debug=False,
        num_devices=N_CORES,
        use_seq_codegen=True,
    )
    xT_d = nc.dram_tensor("xT", [D_IN, j_pad], BF16, kind="ExternalInput")
    wT_d = nc.dram_tensor("wT", [D_IN, D_GOAL], BF16, kind="ExternalInput")
    bias_d = nc.dram_tensor("bias", [P, D_GOAL], F32, kind="ExternalInput")
    srel_d = nc.dram_tensor("srel", [P, R], BF16, kind="ExternalInput")
    out_d = nc.dram_tensor("out", [R, D_GOAL], BF16, kind="ExternalOutput")

    with tile.TileContext(nc) as tc:
        with (
            tc.tile_pool(name="const", bufs=1) as constp,
            tc.tile_pool(name="xs", bufs=6) as xsp,
            tc.tile_pool(name="eab", bufs=4) as eabp,
            tc.tile_pool(name="ost", bufs=4) as ostp,
            tc.tile_pool(name="pmm", bufs=4, space="PSUM") as pmm,
            tc.tile_pool(name="pex", bufs=4, space="PSUM") as pex,
        ):
            xview = xT_d[:].rearrange("(k p) j -> p k j", p=P)

            # HAM warm-up: the PE idles ~12us waiting for the first DMAs and
            # would otherwise spend its first ~3.4us of real work at the cold
            # 1.2GHz clock. Burn that idle window on dummy matmuls instead so
            # the real stream starts at 2.4GHz.
            wu = constp.tile([P, D_GOAL], BF16)
            nc.vector.memset(wu[:], 0.0)
            for wi in range(6):
                psw = pex.tile([P, D_GOAL], F32, tag="ex", name="pswarm")
                nc.tensor.matmul(
                    out=psw[:], lhsT=wu[:, 0:P], rhs=wu[:], start=True, stop=True
                )

            def load_x(pi):
                xg = xsp.tile([P, K_TILES, 2 * P], BF16, tag="xs", name="xgtile")
                nc.sync.dma_start(out=xg[:], in_=xview[:, :, ts(pi, 2 * P)])
                return xg

            # startup-latency critical path: first GEMM matmul (k=0) needs
            # only wt piece 0 + the first half-pair of x, so interleave small
            # wt pieces with the split first x load instead of one 1MB wt DMA
            # ahead of everything
            # wt/bias ride the scalar HWDGE queue so they transfer in
            # parallel with the first x load on the sync queue (stores only
            # reach the scalar queue much later)
            wt = constp.tile([P, K_TILES, D_GOAL], BF16)
            wview = wT_d[:].rearrange("(k p) g -> p k g", p=P)
            nc.scalar.dma_start(out=wt[:, 0:2, :], in_=wview[:, 0:2, :])
            xgs = {0: load_x(0)}
            for kp in range(1, 4):
                nc.scalar.dma_start(
                    out=wt[:, 2 * kp : 2 * kp + 2, :],
                    in_=wview[:, 2 * kp : 2 * kp + 2, :],
                )
            bias = constp.tile([P, D_GOAL], F32)
            nc.scalar.dma_start(out=bias[:], in_=bias_d[:])

            LOOKAHEAD = 5                              # pairs (512KB each)
            for pi in range(1, min(LOOKAHEAD, npairs)):
                xgs[pi] = load_x(pi)

            # srel loaded in pieces, first piece right after the x prefetch
            srel = constp.tile([P, R], BF16)
            srel_loaded = [0]

            def load_srel_piece():
                i = srel_loaded[0]
                if i < NSREL:
                    nc.sync.dma_start(
                        out=srel[:, ts(i, R // NSREL)],
                        in_=srel_d[:, ts(i, R // NSREL)],
                    )
                    srel_loaded[0] = i + 1

            load_srel_piece()

            # iota[p, i, f] = p + 128*i  (plane A: 0..127, plane B: 128..255)
            iota = constp.tile([P, 2, 4 * P], BF16)
            nc.gpsimd.iota(
                iota[:],
                pattern=[[P, 2], [0, 4 * P]],
                base=0,
                channel_multiplier=1,
                allow_small_or_imprecise_dtypes=True,
            )

            yc = constp.tile([P, nchunk, D_GOAL], BF16)

            eabs = {}
            osts = {}
            ncopy = [0]

            def emit_tile(ti):
                bi = ti // 4
                if bi not in eabs:
                    e = eabp.tile([P, 2, 4 * P], BF16, tag="eab", name="etile")
                    nc.vector.tensor_tensor(
                        out=e[:, 0, :],
                        in0=srel[:, ts(bi, 4 * P)],
                        in1=iota[:, 0, :],
                        op=mybir.AluOpType.is_equal,
                    )
                    if any(straddle[4 * bi : 4 * bi + 4]):
                        nc.vector.tensor_tensor(
                            out=e[:, 1, :],
                            in0=srel[:, ts(bi, 4 * P)],
                            in1=iota[:, 1, :],
                            op=mybir.AluOpType.is_equal,
                        )
                    eabs[bi] = e
                e = eabs[bi]
                pso = pex.tile([P, D_GOAL], F32, tag="ex")
                s0 = (ti % 4) * P
                nc.tensor.matmul(
                    out=pso[:],
                    lhsT=e[:, 0, s0 : s0 + P],
                    rhs=yc[:, cc[ti], :],
                    start=True,
                    stop=not straddle[ti],
                )
                if straddle[ti]:
                    nc.tensor.matmul(
                        out=pso[:],
                        lhsT=e[:, 1, s0 : s0 + P],
                        rhs=yc[:, cc[ti] + 1, :],
                        start=False,
                        stop=True,
                    )
                og, oi = divmod(ti, 4)
                if oi == 0:
                    osts[og] = ostp.tile([P, 4, D_GOAL], BF16, tag="ost", name="otile")
                ot = osts[og]
                if ncopy[0] % 2 == 0:
                    nc.scalar.copy(out=ot[:, oi, :], in_=pso[:])
                else:
                    nc.vector.tensor_copy(out=ot[:, oi, :], in_=pso[:])
                ncopy[0] += 1
                if oi == 3:
                    # stores go on the scalar HWDGE queue: they must never
                    # head-of-line-block the x loads on the sync queue. The
                    # final stores are latency-critical (kernel tail): split
                    # them across both queues so they drain in parallel.
                    oview = out_d[ts(og, 4 * P), :].rearrange(
                        "(i p) g -> p i g", p=P
                    )
                    if og >= NB - 2:
                        nc.scalar.dma_start(out=oview[:, 0:2, :], in_=ot[:, 0:2, :])
                        nc.sync.dma_start(out=oview[:, 2:4, :], in_=ot[:, 2:4, :])
                    else:
                        nc.scalar.dma_start(out=oview[:], in_=ot[:])
                    del osts[og]

            ti_next = 0
            for c in range(nchunk):
                pi = c // 2
                if c % 2 == 0:
                    if pi + LOOKAHEAD < npairs:
                        xgs[pi + LOOKAHEAD] = load_x(pi + LOOKAHEAD)
                    if pi in (1, 2, 3, 4, 6, 8, 10):
                        load_srel_piece()
                psy = pmm.tile([P, D_GOAL], F32, tag="mm")
                xg = xgs[pi]
                s0 = (c % 2) * P
                for k in range(K_TILES):
                    nc.tensor.matmul(
                        out=psy[:],
                        lhsT=xg[:, k, s0 : s0 + P],
                        rhs=wt[:, k, :],
                        start=(k == 0),
                        stop=(k == K_TILES - 1),
                    )
                nc.vector.tensor_tensor(
                    out=yc[:, c, :], in0=psy[:], in1=bias[:],
                    op=mybir.AluOpType.add,
                )
                if c % 2 == 1:
                    del xgs[pi]
                while ti_next < NT and (
                    cc[ti_next] + (1 if straddle[ti_next] else 0) + EXPAND_SLACK
                    <= c
                ):
                    emit_tile(ti_next)
                    ti_next += 1
            while srel_loaded[0] < NSREL:
                load_srel_piece()
            while ti_next < NT:
                emit_tile(ti_next)
                ti_next += 1

    nc.compile()
    return nc


_CACHED = {}


def _get_program(nchunk, cc, straddle):
    key = (nchunk, tuple(cc), tuple(straddle))
    if key not in _CACHED:
        _CACHED[key] = build_program(nchunk, cc, straddle)
    return _CACHED[key]


def kernel(x, critic_mask, W, b, _trace=False, **run_kw):
    cc, straddle, nchunk, in_maps = _host_prep(x, critic_mask)
    nc = _get_program(
        nchunk, tuple(int(v) for v in cc), tuple(bool(v) for v in straddle)
    )

    W = np.asarray(W, dtype=np.float32)
    wT = np.ascontiguousarray(W.T).astype(BF)                  # [1024, 512]
    b = np.asarray(b, dtype=np.float32).reshape(1, D_GOAL)
    bias_bc = np.ascontiguousarray(np.broadcast_to(b, (P, D_GOAL)))
    for m in in_maps:
        m["wT"] = wT
        m["bias"] = bias_bc

    res = run_bass_kernel_spmd(
        nc, in_maps, core_ids=list(range(N_CORES)), trace=_trace, **run_kw
    )
    out = np.stack([np.asarray(res.results[c]["out"]) for c in range(N_CORES)])
    out = out.astype(np.float32).reshape(B_FULL, S, D_GOAL)
    if _trace:
        kernel.last_results = res
    return out


if __name__ == "__main__":
    rng = np.random.default_rng(0)
    x = rng.standard_normal((B_FULL, S, D_IN), dtype=np.float32)
    m = rng.integers(0, 2, size=(B_FULL, S)).astype(bool)
    W = rng.standard_normal((D_GOAL, D_IN), dtype=np.float32) / 32.0
    b = rng.standard_normal(D_GOAL).astype(np.float32) * 0.01
    out = kernel(x, m, W, b)
    print(out.shape, out.dtype)


# revision 32
# speedup vs baseline: 1.0062x; 1.0062x over previous
"""Trainium2 Bass kernel for nn_BMManager: Linear([B,S,1024]->[B,S,512]) + bias,
then per-row segment forward-fill (expand_goals).

v3 strategy (data-parallel over batch, 8 cores x 4 batch rows each):

  out[t] = y[idx(t)], y = x @ W^T + b. With a p=0.5 mask only ~half the rows
  are distinct segment starts, so the GEMM runs on COMPACT rows only. No
  device-side gathers (v1 spent 262us of serial GpSimd descriptor-gen):

  Host (numpy): computes the forward-fill index and re-slots the compact
  rows into a *common single-chunk window schedule* shared by all 8 cores:
  every 128-t output tile's sources are placed inside ONE 128-slot chunk
  cc[ti] baked into the program (always feasible: a tile references at most
  128 distinct sources; boundary-shared sources are duplicated, lagging
  cores pad). Uploads x compact pre-transposed bf16 ([1024, j_pad],
  ~18MB/core), W^T bf16, bias broadcast f32, and the per-t relative source
  rank (srcrank_rel in [0,128)) pre-broadcast across partitions in bf16.

  Device, per core:
   1. dense HWDGE DMA loads of x^T (512KB pair-chunk tiles, sync queue).
   2. compact GEMM: per 128-slot chunk, 8 accumulating bf16 matmuls
      (lhsT = x^T slice, rhs = W^T tile) -> PSUM f32; DVE adds bias and
      casts into resident bf16 yc [128, nchunk, 512].
   3. expansion as one-hot matmul: out_tile[t, g] = sum_j E[j, t] yc[j, g],
      exactly ONE matmul per tile (rhs = yc chunk cc[ti]). E built on-device:
      is_equal(srcrank_rel_bcast, iota) on DVE (bf16 2x), one op per 512-t.
   4. DVE/ACT copy PSUM -> bf16 SBUF staging; 512KB stores on the scalar
      HWDGE queue (so stores never head-of-line-block x loads on sync).
      Host upcasts bf16 -> f32.

  PE stream: ~(nchunk*8 + 128) N=512 bf16 matmuls back-to-back (~145us),
  all with FWL-eligible 128-col weights; HAM stays warm.
"""

import numpy as np
import ml_dtypes

import concourse.bacc as bacc
import concourse.mybir as mybir
import concourse.tile as tile
from concourse.bass_utils import run_bass_kernel_spmd

P = 128
N_CORES = 8
B_FULL, S, D_IN, D_GOAL = 32, 4096, 1024, 512
B_PC = B_FULL // N_CORES          # 4 batch rows per core
R = B_PC * S                      # 16384 output rows per core
K_TILES = D_IN // P               # 8
NT = R // P                       # 128 output tiles per core
NB = NT // 4                      # 32 E-build blocks (512 t each)

F32 = mybir.dt.float32
BF16 = mybir.dt.bfloat16
BF = ml_dtypes.bfloat16

EXPAND_SLACK = 4                  # chunks of slack before emitting a tile


def ts(i, n):
    return slice(i * n, (i + 1) * n)


# ---------------------------------------------------------------- host side
def _ffill_index(critic_mask_core):
    """Forward-fill source index per flattened t for one core's 4 rows."""
    mc = np.asarray(critic_mask_core).astype(bool)        # [4, S]
    cond = np.ones((B_PC, S), dtype=bool)
    cond[:, 1:] = mc[:, :-1]
    condf = cond.reshape(-1)                              # [R]
    sel = np.where(condf, np.arange(R), -1)
    idx = np.maximum.accumulate(sel)                      # [R]
    return condf, idx


def _greedy(idx, condf, cc):
    """Place this core's sources into the common window schedule cc.

    All sources of tile ti must land in slots [128*cc[ti], 128*cc[ti]+256).
    Returns (None, (slots_src, srcslot)) on success or (ti, None) on
    overflow.
    """
    srcslot = np.empty(R, np.int64)
    slots_src = []
    s = 0
    last_src = -1
    last_slot = -1
    for ti in range(NT):
        w_lo = P * cc[ti]
        w_hi = w_lo + 2 * P
        if s < w_lo:
            slots_src.extend([0] * (w_lo - s))
            s = w_lo
        t0 = ti * P
        iv = idx[t0 : t0 + P]
        cv = condf[t0 : t0 + P]
        r0 = int(iv[0])
        carried_slot = -1
        if r0 < t0:
            if r0 == last_src and last_slot >= w_lo:
                carried_slot = last_slot
            else:                              # re-place (duplicate) in window
                if s >= w_hi:
                    return ti, None
                carried_slot = s
                slots_src.append(r0)
                s += 1
        new_rs = t0 + np.nonzero(cv)[0]
        k = len(new_rs)
        if s + k > w_hi:
            return ti, None
        base = s
        slots_src.extend(new_rs.tolist())
        s += k
        pos = np.searchsorted(new_rs, iv)
        srcslot[t0 : t0 + P] = np.where(iv < t0, carried_slot, base + pos)
        if k:
            last_src = int(new_rs[-1])
            last_slot = base + k - 1
        elif carried_slot >= 0:
            last_src = r0
            last_slot = carried_slot
    return None, (np.array(slots_src, np.int64), srcslot)


def _schedule(cores):
    """Common two-chunk window schedule cc[ti] + per-core placements."""
    cc = np.zeros(NT, np.int64)
    for condf, idx in cores:
        srcrank = np.cumsum(condf) - 1
        lo = srcrank[idx[np.arange(NT) * P]] // P
        cc = np.maximum(cc, lo)
    cc = np.maximum.accumulate(cc)
    for _ in range(500):
        placements = []
        bad = -1
        for condf, idx in cores:
            ov, res = _greedy(idx, condf, cc)
            if ov is not None:
                bad = max(bad, ov)
                break
            placements.append(res)
        if bad < 0:
            return cc, placements
        cc[bad] += 1
        cc = np.maximum.accumulate(cc)
    raise RuntimeError("window schedule failed to converge")


def _host_prep(x, critic_mask):
    cores = [
        _ffill_index(critic_mask[c * B_PC : (c + 1) * B_PC]) for c in range(N_CORES)
    ]
    cc, placements = _schedule(cores)
    nchunk = int(cc.max()) + 2
    nchunk = -(-nchunk // 2) * 2                       # keep x loads in pairs
    j_pad = nchunk * P

    straddle = np.zeros(NT, bool)
    for _, srcslot in placements:
        hi = srcslot.reshape(NT, P).max(axis=1)
        straddle |= hi >= (cc + 1) * P

    x = np.asarray(x)
    in_maps = []
    for c in range(N_CORES):
        slots_src, srcslot = placements[c]
        slots = np.zeros(j_pad, np.int64)
        slots[: slots_src.size] = slots_src
        xf = x[c * B_PC : (c + 1) * B_PC].reshape(R, D_IN)
        xc = xf[slots]                                  # [j_pad, 1024] f32
        xcT = np.ascontiguousarray(xc.T).astype(BF)     # [1024, j_pad] bf16
        rel = (srcslot - P * cc[np.arange(R) // P]).astype(np.float32)
        assert rel.min() >= 0 and rel.max() < 2 * P
        srel = np.ascontiguousarray(
            np.broadcast_to(rel.astype(BF)[None, :], (P, R))
        )
        in_maps.append({"xT": xcT, "srel": srel})
    return cc, straddle, nchunk, in_maps


# -------------------------------------------------------------- device side
def build_program(nchunk, cc, straddle):
    cc = list(cc)
    straddle = list(straddle)
    j_pad = nchunk * P
    npairs = nchunk // 2
    NSREL = 8                                          # srel load pieces
    nc = bacc.Bacc(
        "TRN2",
        target_bir_lowering=False,
        debug=False,
        num_devices=N_CORES,
        use_seq_codegen=True,
    )
    xT_d = nc.dram_tensor("xT", [D_IN, j_pad], BF16, kind="ExternalInput")
    wT_d = nc.dram_tensor("wT", [D_IN, D_GOAL], BF16, kind="ExternalInput")
    bias_d = nc.dram_tensor("bias", [P, D_GOAL], F32, kind="ExternalInput")
    srel_d = nc.dram_tensor("srel", [P, R], BF16, kind="ExternalInput")
    out_d = nc.dram_tensor("out", [R, D_GOAL], BF16, kind="ExternalOutput")

    with tile.TileContext(nc) as tc:
        with (
            tc.tile_pool(name="const", bufs=1) as constp,
            tc.tile_pool(name="xs", bufs=6) as xsp,
            tc.tile_pool(name="eab", bufs=4) as eabp,
            tc.tile_pool(name="ost", bufs=4) as ostp,
            tc.tile_pool(name="pmm", bufs=4, space="PSUM") as pmm,
            tc.tile_pool(name="pex", bufs=4, space="PSUM") as pex,
        ):
            xview = xT_d[:].rearrange("(k p) j -> p k j", p=P)

            # HAM warm-up: the PE idles ~12us waiting for the first DMAs and
            # would otherwise spend its first ~3.4us of real work at the cold
            # 1.2GHz clock. Burn that idle window on dummy matmuls instead so
            # the real stream starts at 2.4GHz.
            wu = constp.tile([P, D_GOAL], BF16)
            nc.vector.memset(wu[:], 0.0)
            for wi in range(6):
                psw = pex.tile([P, D_GOAL], F32, tag="ex", name="pswarm")
                nc.tensor.matmul(
                    out=psw[:], lhsT=wu[:, 0:P], rhs=wu[:], start=True, stop=True
                )

            def load_x(pi):
                xg = xsp.tile([P, K_TILES, 2 * P], BF16, tag="xs", name="xgtile")
                nc.sync.dma_start(out=xg[:], in_=xview[:, :, ts(pi, 2 * P)])
                return xg

            # startup-latency critical path: first GEMM matmul (k=0) needs
            # only wt piece 0 + the first half-pair of x, so interleave small
            # wt pieces with the split first x load instead of one 1MB wt DMA
            # ahead of everything
            wt = constp.tile([P, K_TILES, D_GOAL], BF16)
            wview = wT_d[:].rearrange("(k p) g -> p k g", p=P)
            nc.sync.dma_start(out=wt[:, 0:2, :], in_=wview[:, 0:2, :])
            xgs = {0: load_x(0)}
            for kp in range(1, 4):
                nc.sync.dma_start(
                    out=wt[:, 2 * kp : 2 * kp + 2, :],
                    in_=wview[:, 2 * kp : 2 * kp + 2, :],
                )
            bias = constp.tile([P, D_GOAL], F32)
            nc.sync.dma_start(out=bias[:], in_=bias_d[:])

            LOOKAHEAD = 5                              # pairs (512KB each)
            for pi in range(1, min(LOOKAHEAD, npairs)):
                xgs[pi] = load_x(pi)

            # srel loaded in pieces, first piece right after the x prefetch
            srel = constp.tile([P, R], BF16)
            srel_loaded = [0]

            def load_srel_piece():
                i = srel_loaded[0]
                if i < NSREL:
                    nc.sync.dma_start(
                        out=srel[:, ts(i, R // NSREL)],
                        in_=srel_d[:, ts(i, R // NSREL)],
                    )
                    srel_loaded[0] = i + 1

            load_srel_piece()

            # iota[p, i, f] = p + 128*i  (plane A: 0..127, plane B: 128..255)
            iota = constp.tile([P, 2, 4 * P], BF16)
            nc.gpsimd.iota(
                iota[:],
                pattern=[[P, 2], [0, 4 * P]],
                base=0,
                channel_multiplier=1,
                allow_small_or_imprecise_dtypes=True,
            )

            yc = constp.tile([P, nchunk, D_GOAL], BF16)

            eabs = {}
            osts = {}
            ncopy = [0]

            def emit_tile(ti):
                bi = ti // 4
                if bi not in eabs:
                    e = eabp.tile([P, 2, 4 * P], BF16, tag="eab", name="etile")
                    nc.vector.tensor_tensor(
                        out=e[:, 0, :],
                        in0=srel[:, ts(bi, 4 * P)],
                        in1=iota[:, 0, :],
                        op=mybir.AluOpType.is_equal,
                    )
                    if any(straddle[4 * bi : 4 * bi + 4]):
                        nc.vector.tensor_tensor(
                            out=e[:, 1, :],
                            in0=srel[:, ts(bi, 4 * P)],
                            in1=iota[:, 1, :],
                            op=mybir.AluOpType.is_equal,
                        )
                    eabs[bi] = e
                e = eabs[bi]
                pso = pex.tile([P, D_GOAL], F32, tag="ex")
                s0 = (ti % 4) * P
                nc.tensor.matmul(
                    out=pso[:],
                    lhsT=e[:, 0, s0 : s0 + P],
                    rhs=yc[:, cc[ti], :],
                    start=True,
                    stop=not straddle[ti],
                )
                if straddle[ti]:
                    nc.tensor.matmul(
                        out=pso[:],
                        lhsT=e[:, 1, s0 : s0 + P],
                        rhs=yc[:, cc[ti] + 1, :],
                        start=False,
                        stop=True,
                    )
                og, oi = divmod(ti, 4)
                if oi == 0:
                    osts[og] = ostp.tile([P, 4, D_GOAL], BF16, tag="ost", name="otile")
                ot = osts[og]
                if ncopy[0] % 2 == 0:
                    nc.scalar.copy(out=ot[:, oi, :], in_=pso[:])
                else:
                    nc.vector.tensor_copy(out=ot[:, oi, :], in_=pso[:])
                ncopy[0] += 1
                if oi == 3:
                    # stores go on the scalar HWDGE queue: they must never
                    # head-of-line-block the x loads on the sync queue. The
                    # final stores are latency-critical (kernel tail): split
                    # them across both queues so they drain in parallel.
                    oview = out_d[ts(og, 4 * P), :].rearrange(
                        "(i p) g -> p i g", p=P
                    )
                    if og >= NB - 2:
                        nc.scalar.dma_start(out=oview[:, 0:2, :], in_=ot[:, 0:2, :])
                        nc.sync.dma_start(out=oview[:, 2:4, :], in_=ot[:, 2:4, :])
                    else:
                        nc.scalar.dma_start(out=oview[:], in_=ot[:])
                    del osts[og]

            ti_next = 0
            for c in range(nchunk):
                pi = c // 2
                if c % 2 == 0:
                    if pi + LOOKAHEAD < npairs:
                        xgs[pi + LOOKAHEAD] = load_x(pi + LOOKAHEAD)
                    if pi in (1, 2, 3, 4, 6, 8, 10):
                        load_srel_piece()
                psy = pmm.tile([P, D_GOAL], F32, tag="mm")
                xg = xgs[pi]
                s0 = (c % 2) * P
                for k in range(K_TILES):
                    nc.tensor.matmul(
                        out=psy[:],
                        lhsT=xg[:, k, s0 : s0 + P],
                        rhs=wt[:, k, :],
                        start=(k == 0),
                        stop=(k == K_TILES - 1),
                    )
                nc.vector.tensor_tensor(
                    out=yc[:, c, :], in0=psy[:], in1=bias[:],
                    op=mybir.AluOpType.add,
                )
                if c % 2 == 1:
                    del xgs[pi]
                while ti_next < NT and (
                    cc[ti_next] + (1 if straddle[ti_next] else 0) + EXPAND_SLACK
                    <= c
                ):
                    emit_tile(ti_next)
                    ti_next += 1
            while srel_loaded[0] < NSREL:
                load_srel_piece()
            while ti_next < NT:
                emit_tile(ti_next)
                ti_next += 1

    nc.compile()
    return nc


_CACHED = {}


def _get_program(nchunk, cc, straddle):
    key = (nchunk, tuple(cc), tuple(straddle))
    if key not in _CACHED:
        _CACHED[key] = build_program(nchunk, cc, straddle)
    return _CACHED[key]


def kernel(x, critic_mask, W, b, _trace=False, **run_kw):
    cc, straddle, nchunk, in_maps = _host_prep(x, critic_mask)
    nc = _get_program(
        nchunk, tuple(int(v) for v in cc), tuple(bool(v) for v in straddle)
    )

    W = np.asarray(W, dtype=np.float32)
    wT = np.ascontiguousarray(W.T).astype(BF)                  # [1024, 512]
    b = np.asarray(b, dtype=np.float32).reshape(1, D_GOAL)
    bias_bc = np.ascontiguousarray(np.broadcast_to(b, (P, D_GOAL)))
    for m in in_maps:
        m["wT"] = wT
        m["bias"] = bias_bc

    res = run_bass_kernel_spmd(
        nc, in_maps, core_ids=list(range(N_CORES)), trace=_trace, **run_kw
    )
    out = np.stack([np.asarray(res.results[c]["out"]) for c in range(N_CORES)])
    out = out.astype(np.float32).reshape(B_FULL, S, D_GOAL)
    if _trace:
        kernel.last_results = res
    return out


if __name__ == "__main__":
    rng = np.random.default_rng(0)
    x = rng.standard_normal((B_FULL, S, D_IN), dtype=np.float32)
    m = rng.integers(0, 2, size=(B_FULL, S)).astype(bool)
    W = rng.standard_normal((D_GOAL, D_IN), dtype=np.float32) / 32.0
    b = rng.standard_normal(D_GOAL).astype(np.float32) * 0.01
    out = kernel(x, m, W, b)
    print(out.shape, out.dtype)
